# revision 28
# baseline (speedup 1.0000x reference)
"""Trainium2 Bass kernel for nn_KernelAxialMultiAttention (linear attention).

Math (per independent (b, m) slice; x: [T=256, C=512], N=8 heads, D=64):
  q = elu(x @ Wq.T) + 1          [T, C]   (heads along C)
  k = elu(x @ Wk.T) + 1
  ksum[c]   = sum_t k[t, c]
  krow[n,t] = sum_{c in head n} k[t, c]
  zden[n,t] = sum_{c in head n} q[t, c] * ksum[c];  z = 1/zden
  s[n, c]   = sum_t krow[n, t] * x[t, c]
  u[n, e]   = sum_c s[n, c] * Wv[n*D+e, c]     (= KtV column sums)
  w[n, cO]  = sum_e u[n, e] * Wp[cO, n*D+e]
  out[t,cO] = sum_n z[n, t] * w[n, cO]
This is algebraically identical to the reference (sum reordering only); the
v-projection and output projection collapse because Z is constant over D.

elu(x)+1 is computed as relu(x) + min(exp(x), 1):
  k tiles: exp on Scalar; min and (relu + add + ksum accum) on Vector.
  q tiles: exp and relu on Scalar; fused (min + add) on Vector.

zden uses per-slice wz = maskT*ksum (one broadcast multiply on Vector) as the
cheap 8-col stationary; zden for 4 slices accumulates into one PSUM tile at
32-aligned partition stripes, matching the quad layout of z_all/w4q so the
final out-matmuls hit distinct PE row groups (tile_position=(32j, 0)).
The w-projection batches 4 slices through one gm4 [128, 128] stationary
(mask-padded to 32-col stripes). Phase B/C run per quad, interleaved after
every other pair, so only the last quad's epilogue is exposed.
All matmul operands are bf16 (fp32 PSUM accumulation); the output is
returned as bf16 and upcast on the host.

Sharding: data-parallel over the 128 (b, m) slices -> 16 per NeuronCore.
"""

import os
import sys

import numpy as np

for _p in ("/opt/trn_rl_repo", "/root/.axon_site/_ro/trn_rl_repo"):
    if os.path.isdir(_p) and _p not in sys.path:
        sys.path.insert(0, _p)

B, M, T, C = 2, 64, 256, 512
NH, D = 8, 64
S = 16            # slices per core
NCORES = 8
P = 128           # partitions
NKC = C // P      # 4 contraction chunks
NTC = T // P      # 2 t chunks
SG = 8            # slices per phase-B/C group

_BUILT = {}


def _build_nc():
    from contextlib import ExitStack

    import concourse.bacc as bacc
    import concourse.bass as bass
    import concourse.mybir as mybir
    import concourse.tile as tile

    f32 = mybir.dt.float32
    bf16 = mybir.dt.bfloat16
    AF = mybir.ActivationFunctionType
    OP = mybir.AluOpType

    nc = bacc.Bacc(None, target_bir_lowering=False)
    x_d = nc.declare_dram_parameter("x16", [S, T, C], bf16, isOutput=False)
    xT_d = nc.declare_dram_parameter("xT16", [S, C, T], bf16, isOutput=False)
    wqT_d = nc.declare_dram_parameter("WqT16", [C, C], bf16, isOutput=False)
    wkT_d = nc.declare_dram_parameter("WkT16", [C, C], bf16, isOutput=False)
    wvT_d = nc.declare_dram_parameter("WvT", [C, C], bf16, isOutput=False)
    wpT_d = nc.declare_dram_parameter("WpT", [C, C], bf16, isOutput=False)
    out_d = nc.declare_dram_parameter("out", [S, T, C], bf16, isOutput=True)

    with tile.TileContext(nc) as tc, ExitStack() as ctx:
        wpool = ctx.enter_context(tc.tile_pool(name="weights", bufs=1))
        cpool = ctx.enter_context(tc.tile_pool(name="consts", bufs=1))
        persist = ctx.enter_context(tc.tile_pool(name="persist", bufs=1))
        xn_pool = ctx.enter_context(tc.tile_pool(name="xnat", bufs=4))
        xt_pool = ctx.enter_context(tc.tile_pool(name="xT", bufs=3))
        e_pool = ctx.enter_context(tc.tile_pool(name="etile", bufs=3))
        cl_pool = ctx.enter_context(tc.tile_pool(name="ctile", bufs=3))
        qe_pool = ctx.enter_context(tc.tile_pool(name="qe", bufs=3))
        ke_pool = ctx.enter_context(tc.tile_pool(name="ke", bufs=3))
        ksum_pool = ctx.enter_context(tc.tile_pool(name="ksum", bufs=2))
        wz_pool = ctx.enter_context(tc.tile_pool(name="wz", bufs=2))
        zq_pool = ctx.enter_context(tc.tile_pool(name="zq", bufs=2))
        krt_pool = ctx.enter_context(tc.tile_pool(name="krowT", bufs=2))
        gm_pool = ctx.enter_context(tc.tile_pool(name="gm", bufs=2))
        w4_pool = ctx.enter_context(tc.tile_pool(name="w4", bufs=2))
        osb_pool = ctx.enter_context(tc.tile_pool(name="outsb", bufs=3))

        ps_proj = ctx.enter_context(
            tc.tile_pool(name="ps_proj", bufs=3, space=bass.MemorySpace.PSUM))
        ps_small = ctx.enter_context(
            tc.tile_pool(name="ps_small", bufs=2, space=bass.MemorySpace.PSUM))
        ps_z = ctx.enter_context(
            tc.tile_pool(name="ps_z", bufs=1, space=bass.MemorySpace.PSUM))
        ps_out = ctx.enter_context(
            tc.tile_pool(name="ps_out", bufs=2, space=bass.MemorySpace.PSUM))

        # ---- weights (host-pretransposed) into SBUF ----
        # layout [c % 128, c // 128, row]
        wqT = wpool.tile([P, NKC, C], bf16, tag="wqT")
        wkT = wpool.tile([P, NKC, C], bf16, tag="wkT")
        wvT = wpool.tile([P, NKC, C], bf16, tag="wvT")
        wpT = wpool.tile([P, NKC, C], bf16, tag="wpT")
        for kc in range(NKC):
            nc.gpsimd.dma_start(
                out=wkT[:, kc, :],
                in_=wkT_d.rearrange("(a p) d -> p a d", p=P)[:, kc, :])

        # ---- head-block masks: maskT[:, ci, n] = 1 if (128*ci + p)//64 == n ----
        maskT = cpool.tile([P, NKC, NH], bf16, tag="maskT")
        # maskE: same but f32 and padded to 32 cols (cols 8..31 stay zero)
        maskE = cpool.tile([P, NKC, 32], f32, tag="maskE")

        def _build_masks():
            nc.gpsimd.memset(maskT[:], 0.0)
            nc.gpsimd.memset(maskE[:], 0.0)
            for ci in range(NKC):
                nc.gpsimd.memset(maskT[0:64, ci, 2 * ci:2 * ci + 1], 1.0)
                nc.gpsimd.memset(maskT[64:128, ci, 2 * ci + 1:2 * ci + 2], 1.0)
                nc.gpsimd.memset(maskE[0:64, ci, 2 * ci:2 * ci + 1], 1.0)
                nc.gpsimd.memset(maskE[64:128, ci, 2 * ci + 1:2 * ci + 2], 1.0)

        sT_all = persist.tile([P, NKC, S, NH], bf16, tag="sT_all")
        # z for slice s lives at partitions 32*(s%4)..+8, chunk s//4
        z_all = persist.tile([P, S // 4, T], bf16, tag="z_all")
        uT_sb = persist.tile([P, NKC, S], f32, tag="uT_sb")

        # ---- HAM pre-warm: dummy matmuls fill the DMA-wait window so the
        # PE clock gate is at 8/8 when the first real projection starts.
        warm = cpool.tile([P, P], bf16, tag="warm")
        nc.vector.memset(warm[:], 0.0)
        for _ in range(40):
            wps = ps_proj.tile([P, P], f32, tag="proj")
            nc.tensor.matmul(wps[:], warm[:], warm[:], start=True, stop=True)

        x3 = x_d  # [S, T, C] bf16
        zden_hold = [None]

        if True:
            # =============== phase A: per-pair projections ==================
            for p in range(S // 2):
                s0, s1 = 2 * p, 2 * p + 1
                xn = []
                for s in (s0, s1):
                    t_ = xn_pool.tile([P, NTC, C], bf16, tag="xnat")
                    nc.gpsimd.dma_start(
                        out=t_[:],
                        in_=x3[s].rearrange("(a p) c -> p a c", p=P),
                    )
                    xn.append(t_)
                xT = xt_pool.tile([P, NKC, 2, T], bf16, tag="xT")
                if p == 0:
                    for kcc in range(NKC):
                        for si, s in ((0, s0), (1, s1)):
                            nc.sync.dma_start(
                                out=xT[:, kcc, si, :],
                                in_=xT_d[s].rearrange(
                                    "(a p) t -> p a t", p=P)[:, kcc, :],
                            )
                else:
                    for si, s in ((0, s0), (1, s1)):
                        nc.sync.dma_start(
                            out=xT[:, :, si, :],
                            in_=xT_d[s].rearrange("(a p) t -> p a t", p=P),
                        )
                if p == 0:
                    _build_masks()
                    for wT, wd in ((wqT, wqT_d), (wvT, wvT_d), (wpT, wpT_d)):
                        nc.sync.dma_start(
                            out=wT[:],
                            in_=wd.rearrange("(a p) d -> p a d", p=P))

                ksum = ksum_pool.tile([P, NKC, 2], f32, tag="ksum")
                ke = ke_pool.tile([P, NKC, 2 * T], bf16, tag="ke")
                qe = qe_pool.tile([P, NKC, 2 * T], bf16, tag="qe")

                # ---- k projection first (produces ksum); then krowT/sT
                # (the long pole feeding phase B/C) ahead of the q
                # projection in the tensor queue; then q and the z path.
                def _proj(wT, etile, is_k):
                    for mc in range(NKC):
                        pp = ps_proj.tile([P, 2 * T], f32, tag="proj")
                        for kc in range(NKC):
                            nc.tensor.matmul(
                                pp[:],
                                wT[:, kc, mc * P:(mc + 1) * P],
                                xT[:, kc, :, :],
                                start=(kc == 0),
                                stop=(kc == NKC - 1),
                            )
                        et = e_pool.tile([P, 2 * T], bf16, tag="etile")
                        nc.scalar.activation(et[:], pp[:], AF.Exp)
                        if is_k:
                            ct = cl_pool.tile([P, 2 * T], bf16, tag="ctile")
                            nc.vector.tensor_scalar_min(ct[:], et[:], 1.0)
                            for h in range(2):
                                nc.vector.scalar_tensor_tensor(
                                    etile[:, mc, h * T:(h + 1) * T],
                                    pp[:, h * T:(h + 1) * T], 0.0,
                                    ct[:, h * T:(h + 1) * T],
                                    OP.max, OP.add,
                                    accum_out=ksum[:, mc, h:h + 1])
                        else:
                            rt = cl_pool.tile([P, 2 * T], bf16, tag="ctile")
                            nc.scalar.activation(rt[:], pp[:], AF.Relu)
                            nc.vector.scalar_tensor_tensor(
                                etile[:, mc, :], et[:], 1.0, rt[:],
                                OP.min, OP.add)

                _proj(wkT, ke, True)

                for si, s in ((0, s0), (1, s1)):
                    # krowT[t, n] = sum_c maskT[c, n] * ke[c, t]
                    krt_ps = ps_small.tile([P, NTC, NH], f32, tag="sm")
                    for tcb in range(NTC):
                        for mc in range(NKC):
                            nc.tensor.matmul(
                                krt_ps[:, tcb, :],
                                ke[:, mc,
                                   si * T + tcb * P: si * T + (tcb + 1) * P],
                                maskT[:, mc, :],
                                start=(mc == 0),
                                stop=(mc == NKC - 1),
                            )
                    krt = krt_pool.tile([P, NTC, NH], bf16, tag="krt")
                    nc.vector.tensor_copy(krt[:], krt_ps[:])

                    # sT[c, n] = sum_t x[t, c] * krowT[t, n]
                    st_ps = ps_small.tile([P, NKC, NH], f32, tag="sm")
                    for mc in range(NKC):
                        for tcb in range(NTC):
                            nc.tensor.matmul(
                                st_ps[:, mc, :],
                                xn[si][:, tcb, mc * P:(mc + 1) * P],
                                krt[:, tcb, :],
                                start=(tcb == 0),
                                stop=(tcb == NTC - 1),
                            )
                    nc.vector.tensor_copy(sT_all[:, :, s, :], st_ps[:])

                _proj(wqT, qe, False)

                # ---- wz = maskT * ksum (per slice), zden, z ----
                # zden for slice s accumulates at psum partitions 32*(s%4).
                wz = wz_pool.tile([P, NKC, 2, NH], bf16, tag="wz")
                for si in range(2):
                    nc.vector.tensor_tensor(
                        out=wz[:, :, si, :], in0=maskE[:, :, 0:NH],
                        in1=ksum[:, :, si, None].broadcast_to((P, NKC, NH)),
                        op=OP.mult)
                if p % 2 == 0:
                    zden_q = ps_z.tile([P, T], f32, tag="zden")
                    zden_hold[0] = zden_q
                else:
                    zden_q = zden_hold[0]
                for si, s in ((0, s0), (1, s1)):
                    j = 32 * (s % 4)
                    for mc in range(NKC):
                        nc.tensor.matmul(
                            zden_q[j:j + NH, :],
                            wz[:, mc, si, :],
                            qe[:, mc, si * T:(si + 1) * T],
                            start=(mc == 0),
                            stop=(mc == NKC - 1),
                            tile_position=(0, j),
                        )
                if p % 2 == 1:
                    # batched reciprocal for the whole quad (unused stripes
                    # hold garbage; they are never read)
                    zq = zq_pool.tile([P, T], f32, tag="zq")
                    nc.vector.reciprocal_approx_fast(zq[:], zden_q[:])
                    nc.vector.tensor_copy(z_all[:, p // 2, :], zq[:])

                if p % 2 == 0:
                    continue
                qd = p // 2
                # ========= phase B: u for this quad =========================
                # uT[e, n, slice] = sum_c WvT[c, n*D+e] * sT[c, n, slice]
                ut_ps = ps_small.tile([P, NKC, 4], f32, tag="sm")
                for n in range(NH):
                    r0 = 64 * (n % 2)
                    for kc in range(NKC):
                        nc.tensor.matmul(
                            ut_ps[r0:r0 + 64, n // 2, :],
                            wvT[:, kc, n * D:(n + 1) * D],
                            sT_all[:, kc, 4 * qd:4 * (qd + 1), n],
                            start=(kc == 0),
                            stop=(kc == NKC - 1),
                        )
                nc.scalar.copy(uT_sb[:, :, 4 * qd:4 * (qd + 1)], ut_ps[:])

                # ========= phase C: w (batched per quad), out ===============
                # gm4[c, ci, 32*j + n] = maskE[c, ci, n] * uT[c, ci, s];
                # 24 pad columns per slice stay zero (maskE is zero there).
                gm4 = gm_pool.tile([P, NKC, P], bf16, tag="gm4")
                for j in range(4):
                    s = 4 * qd + j
                    nc.vector.tensor_tensor(
                        out=gm4[:, :, 32 * j:32 * (j + 1)], in0=maskE[:],
                        in1=uT_sb[:, :, s, None].broadcast_to((P, NKC, 32)),
                        op=OP.mult)
                # w_q[32*j + n, cO] = sum_c gm4[c, ci, 32*j+n] * WpT[c, cO]
                w_ps = ps_small.tile([P, C], f32, tag="sm")
                for ci in range(NKC):
                    nc.tensor.matmul(
                        w_ps[:],
                        gm4[:, ci, :],
                        wpT[:, ci, :],
                        start=(ci == 0),
                        stop=(ci == NKC - 1),
                    )
                w4q = w4_pool.tile([P, C], bf16, tag="w4q")
                nc.scalar.copy(w4q[:], w_ps[:])

                for j in range(4):
                    s = 4 * qd + j
                    osb = osb_pool.tile([P, NTC, C], bf16, tag="outsb")
                    for tcb in range(NTC):
                        o_ps = ps_out.tile([P, C], f32, tag="o_ps")
                        nc.tensor.matmul(
                            o_ps[:],
                            z_all[32 * j:32 * j + NH, qd, tcb * P:(tcb + 1) * P],
                            w4q[32 * j:32 * j + NH, :],
                            start=True,
                            stop=True,
                            tile_position=(32 * j, 0),
                        )
                        if tcb == 0:
                            nc.scalar.copy(osb[:, tcb, :], o_ps[:])
                        else:
                            nc.vector.tensor_copy(osb[:, tcb, :], o_ps[:])
                        eng = nc.gpsimd if tcb == 0 else nc.sync
                        eng.dma_start(
                            out=out_d[s, tcb * P:(tcb + 1) * P, :],
                            in_=osb[:, tcb, :],
                        )

    nc.compile()
    return nc


def _get_nc():
    if "nc" not in _BUILT:
        _BUILT["nc"] = _build_nc()
    return _BUILT["nc"]


def kernel(**inputs):
    import ml_dtypes

    bf16 = ml_dtypes.bfloat16
    x = np.asarray(inputs["x"], dtype=np.float32)
    Wq = np.asarray(inputs["Wq"], dtype=np.float32)
    Wk = np.asarray(inputs["Wk"], dtype=np.float32)
    Wv = np.asarray(inputs["Wv"], dtype=np.float32)
    Wp = np.asarray(inputs["Wp"], dtype=np.float32)
    bp = np.asarray(inputs.get("bp", np.zeros(C)), dtype=np.float32)

    x16 = np.ascontiguousarray(x.reshape(B * M, T, C).astype(bf16))
    xT16 = np.ascontiguousarray(x16.transpose(0, 2, 1))
    wqT16 = np.ascontiguousarray(Wq.T.astype(bf16))
    wkT16 = np.ascontiguousarray(Wk.T.astype(bf16))
    wvT = np.ascontiguousarray(Wv.T.astype(bf16))
    wpT = np.ascontiguousarray(Wp.T.astype(bf16))
    in_maps = []
    for i in range(NCORES):
        in_maps.append({
            "x16": np.ascontiguousarray(x16[S * i:S * (i + 1)]),
            "xT16": np.ascontiguousarray(xT16[S * i:S * (i + 1)]),
            "WqT16": wqT16, "WkT16": wkT16, "WvT": wvT, "WpT": wpT,
        })

    from concourse.bass_utils import run_bass_kernel_spmd

    nc = _get_nc()
    trace = os.environ.get("KERNEL_TRACE", "0") == "1"
    res = run_bass_kernel_spmd(nc, in_maps, list(range(NCORES)), trace=trace)
    if trace and res.exec_time_ns is not None:
        print(f"HW exec time: {res.exec_time_ns} ns", flush=True)
        _BUILT["exec_time_ns"] = res.exec_time_ns

    out = np.concatenate([res.results[i]["out"] for i in range(NCORES)], axis=0)
    out = out.astype(np.float32).reshape(B, M, T, C)
    if np.any(bp):
        out = out + bp
    return out
